# revision 39
# baseline (speedup 1.0000x reference)
"""Multi-head graph attention layer (GAT) for Trainium2, 8-core data-parallel.

Problem: B=8, N=1024, D_IN=256, D_OUT=64, H=8, LeakyReLU slope 0.2.
  Wh = einsum('bnd,hdf->bhnf', h, W)
  f1 = Wh @ a1, f2 = Wh @ a2              (per head)
  e  = leaky_relu(f1[:,None] + f2[None,:])
  att = softmax(where(adj==0, -inf, e))
  out = att @ Wh  -> concat heads [B, N, H*F]

Sharding: one batch element per NeuronCore (B=8 across 8 cores).

Algebra: with x = f1_i + f2_j,
  exp(leaky_relu(x)) = max(exp(x), exp(0.2 x))
                     = E1s_i * max(R_i * E2_j, E2s_j)
where R = exp(0.8 f1), E2 = exp(f2), E2s = exp(0.2 f2), E1s = exp(0.2 f1).
The E1s_i factor is constant along j, so it cancels between the softmax
numerator and denominator and is never computed.  Per (head, j-tile) the
[128, N] unnormalized attention U' = max(R_i*E2_j, E2s_j) * adj_ij needs
only TWO elementwise ops:
  1. a dual-op tensor_scalar (mult by per-partition E2_j, max with
     per-partition E2s_j) streaming the broadcast R row — 4x DVE mode;
  2. a tensor_tensor multiply with the adjacency tile, two heads per
     instruction (2x DVE mode).
U' accumulates against [Wh | 1] so column 64 of out^T is the softmax
denominator Z; normalization happens after a PE transpose.

Scheduling notes (from HW traces):
- GPSIMD streaming steals SBUF ports and collapses DVE's 4x
  tensor_scalar mode to 1x (443ns -> 2260ns), so Pool gets no [N,N] work.
- Each dma_start costs ~640ns of serial descriptor-gen on its issuing
  engine and input DMAs compete for HBM bandwidth; input gens are
  serialized on sync in first-use order, output gens on gpsimd.
- The R row broadcast is done by PE outer products against a host-sent
  selector (sel_h @ e12t replicates row h across partitions) + ACT
  evacuation - no DMA round-trip, and it rides the high-priority chain
  that gates the whole DVE stream.
- Tile deps are whole-tile: rall is 8 separate tiles, and the output
  lives in column-half strips so the heads 0-3 half DMAs out mid-run;
  the two drain-head finalizes run concurrently on disjoint engines.
"""

import numpy as np
import ml_dtypes

BF16 = ml_dtypes.bfloat16

B, N, D_IN, D_OUT, H = 8, 1024, 256, 64, 8
NEG_SLOPE = 0.2
P = 128                       # partitions
NJT = N // P                  # 8 j-tiles
NIT = N // P                  # 8 i-tiles
NKT = D_IN // P               # 2 k-tiles
HF = H * D_OUT                # 512
AUG = D_OUT + 1               # 65 (Wh columns + ones column)
FT_W = 40                     # f-score width: f1 cols 0:8, f2 cols 32:40
ACT_JTS = (6,)                 # j-tiles whose max() runs on ACT via the
                              # relu identity (must avoid jt 0 and 7)


def _build_program():
    """Build the single-core SPMD Bass program. Returns nc."""
    import concourse.bass as bass
    import concourse.bacc as bacc
    import concourse.tile as tile
    from concourse import mybir
    from concourse.masks import make_identity

    f32 = mybir.dt.float32
    bf16 = mybir.dt.bfloat16
    AF = mybir.ActivationFunctionType
    OP = mybir.AluOpType

    nc = bacc.Bacc("TRN2", target_bir_lowering=False, debug=False,
                   enable_asserts=False, num_devices=8)

    hT = nc.dram_tensor("hT", [D_IN, N], bf16, kind="ExternalInput").ap()
    adjT = nc.dram_tensor("adjT", [N, N], bf16, kind="ExternalInput").ap()
    wrs = nc.dram_tensor("wrs", [D_IN, HF], bf16, kind="ExternalInput").ap()
    w12 = nc.dram_tensor("w12", [D_IN, FT_W], bf16,
                         kind="ExternalInput").ap()
    sel = nc.dram_tensor("sel", [H, H * P], bf16, kind="ExternalInput").ap()
    out = nc.dram_tensor("out", [N, HF], bf16, kind="ExternalOutput").ap()

    with tile.TileContext(nc) as tc:
        with (
            tc.tile_pool(name="const", bufs=1) as const,
            tc.tile_pool(name="inputs", bufs=1) as inputs,
            tc.tile_pool(name="whp", bufs=1) as whp,
            tc.tile_pool(name="ecol", bufs=1) as ecolp,
            tc.tile_pool(name="ps_s", bufs=2, space="PSUM") as ps_s,
            tc.tile_pool(name="ps_ot", bufs=2, space="PSUM") as ps_ot,
            tc.tile_pool(name="ps_tr", bufs=1, space="PSUM") as ps_tr,
            tc.tile_pool(name="work", bufs=3) as work,
            tc.tile_pool(name="fin", bufs=3) as fin,
        ):
            # ---- Phase 0: input loads -------------------------------------
            # ht/w12 gate the R-broadcast chain: their descriptor-gen goes
            # first on sync.  adj/wrs gen on gpsimd (behind make_identity,
            # which is fine - first use is later).
            # All input descriptor-gen serialized on sync in priority order,
            # so hT's transfers hit clean DMA queues before adj's 2MB.
            ht_sb = []
            for kt in range(NKT):
                t = inputs.tile([P, N], bf16, tag=f"ht{kt}")
                for c in range(2):
                    nc.sync.dma_start(
                        out=t[:, c * 512:(c + 1) * 512],
                        in_=hT[kt * P:(kt + 1) * P, c * 512:(c + 1) * 512])
                ht_sb.append(t)
            w12_sb = []
            for kt in range(NKT):
                t = inputs.tile([P, FT_W], bf16, tag=f"w12{kt}")
                nc.sync.dma_start(out=t, in_=w12[kt * P:(kt + 1) * P, :])
                w12_sb.append(t)
            selsb = const.tile([H, H, P], bf16)
            nc.sync.dma_start(out=selsb, in_=sel)
            adj_sb = []
            for jt in range(NJT):
                t = inputs.tile([P, N], bf16, tag=f"adj{jt}")
                nc.sync.dma_start(out=t, in_=adjT[jt * P:(jt + 1) * P, :])
                adj_sb.append(t)
            wrs_sb = []
            for kt in range(NKT):
                t = inputs.tile([P, HF], bf16, tag=f"wrs{kt}")
                nc.sync.dma_start(out=t, in_=wrs[kt * P:(kt + 1) * P, :])
                wrs_sb.append(t)

            ident = const.tile([P, P], f32)
            make_identity(nc, ident)

            # ---- Phase 1a (critical chain): R broadcast -------------------
            # ft[h, :] = f1_h (transposed);  e12t = exp(0.8 f1)
            with tc.high_priority():
                ft = ps_tr.tile([H, N], f32, tag='big')
                for nh in range(2):
                    for kt in range(NKT):
                        nc.tensor.matmul(ft[:, nh * 512:(nh + 1) * 512],
                                         w12_sb[kt][:, 0:H],
                                         ht_sb[kt][:, nh * 512:(nh + 1) * 512],
                                         start=(kt == 0),
                                         stop=(kt == NKT - 1))
                e12t = const.tile([H, N], bf16)
                nc.scalar.activation(e12t, ft, AF.Exp, scale=1.0 - NEG_SLOPE)
                # broadcast R rows across partitions via PE outer products
                # (ones_row^T @ row) + ACT evacuation - no DMA involved
                rall = []
                for h in range(H):
                    op = ps_tr.tile([P, N], f32, tag='big')
                    for nh in range(2):
                        nc.tensor.matmul(op[:, nh * 512:(nh + 1) * 512],
                                         selsb[:, h, :],
                                         e12t[:, nh * 512:(nh + 1) * 512],
                                         start=True, stop=True)
                    rt = const.tile([P, N], bf16, tag=f"rall{h}")
                    nc.scalar.activation(rt, op, AF.Copy, scale=1.0)
                    rall.append(rt)

            # ---- Phase 1b: per-j-tile scalars E2/E2s ----------------------
            # ecols[jt][:, h] = E2_j = exp(f2_h,j); [:, 32+h] = E2s_j
            ecols = []
            for jt in range(NJT):
                psec = ps_s.tile([P, FT_W], f32, tag='pss')
                for kt in range(NKT):
                    nc.tensor.matmul(psec, ht_sb[kt][:, jt * P:(jt + 1) * P],
                                     w12_sb[kt],
                                     start=(kt == 0), stop=(kt == NKT - 1))
                ec = ecolp.tile([P, FT_W], f32, tag=f"ecols{jt}")
                nc.scalar.activation(ec[:, 0:H], psec[:, 32:32 + H], AF.Exp,
                                     scale=1.0)
                nc.scalar.activation(ec[:, 32:32 + H], psec[:, 32:32 + H],
                                     AF.Exp, scale=NEG_SLOPE)
                if jt in ACT_JTS:   # negated E2s as ACT relu bias
                    nc.vector.tensor_scalar_mul(ec[:, H:2 * H],
                                                ec[:, 32:32 + H], -1.0)
                ecols.append(ec)

            # ---- Phase 2: whaug = [Wh | 1] (no per-head scaling) ----------
            whaug = []
            for jt in range(NJT):
                ps = ps_s.tile([P, HF], f32, tag='pss')
                for kt in range(NKT):
                    lhsT = ht_sb[kt][:, jt * P:(jt + 1) * P]
                    nc.tensor.matmul(ps, lhsT, wrs_sb[kt],
                                     start=(kt == 0), stop=(kt == NKT - 1))
                wa = whp.tile([P, H, AUG], bf16, tag=f"whaug{jt}")
                nc.scalar.activation(wa[:, :, 0:D_OUT], ps, AF.Copy,
                                     scale=1.0)
                nc.vector.memset(wa[:, :, D_OUT], 1.0)
                whaug.append(wa)
            whe2s = {}
            for jt in ACT_JTS:
                we = whp.tile([P, H, AUG], bf16, tag=f"whe{jt}")
                for h in range(H):
                    nc.scalar.activation(we[:, h, :], whaug[jt][:, h, :],
                                         AF.Copy,
                                         scale=ecols[jt][:, 32 + h:33 + h])
                whe2s[jt] = we

            # output strips: column halves per i-tile (512B DMA lines); the
            # heads 0-3 half leaves mid-run, only the hi half is on the tail
            out_lo = []
            for it in range(NIT):
                osl = whp.tile([P, HF // 2], bf16, tag=f"osbl{it}")
                out_lo.append(osl)
            out_hi = []
            for t2 in range(NIT // 2):
                osh = whp.tile([P, 2, HF // 2], bf16, tag=f"osbh{t2}")
                out_hi.append(osh)

            # pre-built ACT-road pt tiles: relu(E2*R - E2s); the E2s*adj
            # remainder is added back by PE matmuls streaming adj directly
            pta = {}
            for hp in range(H // 2):
                for jt in ACT_JTS:
                    t = work.tile([P, 2, N], bf16, tag=f"pta{hp}_{jt}",
                                  bufs=1)
                    for ph in range(2):
                        h = 2 * hp + ph
                        nc.scalar.activation(
                            t[:, ph, :], rall[h], AF.Relu,
                            scale=ecols[jt][:, h:h + 1],
                            bias=ecols[jt][:, H + h:H + h + 1])
                    pta[(hp, jt)] = t

            # ---- Phase 3: per-head-pair attention -------------------------
            def _dst(h, it):
                if h < 4:
                    return out_lo[it][:, h * D_OUT:(h + 1) * D_OUT]
                return out_hi[it // 2][:, it % 2,
                                       (h - 4) * D_OUT:(h - 3) * D_OUT]

            def finalize(hp, h, ot, muls_on_dve):
                ots = fin.tile([AUG, N], f32, tag="ots")
                nc.scalar.copy(ots, ot)
                if muls_on_dve:
                    # drain head: per-it pipeline on pss ring + DVE, fully
                    # concurrent with the sibling head's ACT-road finalize
                    for it in range(NIT):
                        tr2 = ps_s.tile([P, AUG], f32, tag='pss')
                        nc.tensor.transpose(tr2, ots[:, it * P:(it + 1) * P],
                                            ident[0:AUG, 0:AUG])
                        rcol = fin.tile([P, 1], f32, tag="rcol")
                        nc.vector.reciprocal(rcol, tr2[:, D_OUT:AUG])
                        nc.vector.tensor_scalar_mul(_dst(h, it),
                                                    tr2[:, 0:D_OUT], rcol)
                    return
                # stride-128 slots keep each transpose in one PSUM bank
                tr2all = ps_tr.tile([P, NIT, P], f32, tag='big')
                for it in range(NIT):
                    nc.tensor.transpose(tr2all[:, it, 0:AUG],
                                        ots[:, it * P:(it + 1) * P],
                                        ident[0:AUG, 0:AUG])
                rcall = fin.tile([P, NIT], f32, tag="rcall")
                nc.vector.reciprocal(rcall, tr2all[:, :, D_OUT])
                for it in range(NIT):
                    nc.scalar.activation(_dst(h, it), tr2all[:, it, 0:D_OUT],
                                         AF.Copy, scale=rcall[:, it:it + 1])

            for hp in range(H // 2):
                h0, h1 = 2 * hp, 2 * hp + 1
                ot0 = ps_ot.tile([AUG, N], f32, tag="ot")
                ot1 = ps_ot.tile([AUG, N], f32, tag="ot")
                for jt in range(NJT):
                    ec = ecols[jt]
                    if jt in ACT_JTS:
                        pt2 = pta[(hp, jt)]
                    else:
                        # pt = max(R_i*E2_j, E2s_j)  (dual-op ts, 4x DVE)
                        pt2 = work.tile([P, 2, N], bf16, tag="pt", bufs=6)
                        for ph, h in ((0, h0), (1, h1)):
                            nc.vector.tensor_scalar(
                                out=pt2[:, ph, :], in0=rall[h],
                                scalar1=ec[:, h:h + 1],
                                scalar2=ec[:, 32 + h:32 + h + 1],
                                op0=OP.mult, op1=OP.max)
                    # um = pt * adj   (both heads in one 2x DVE pass)
                    um2 = work.tile([P, 2, N], bf16, tag="um", bufs=10)
                    nc.vector.tensor_tensor(
                        out=um2, in0=pt2,
                        in1=adj_sb[jt].unsqueeze(1).broadcast_to([P, 2, N]),
                        op=OP.mult)
                    # out^T[(f|1), i] += [Wh | 1]^T @ um
                    for ph, h, ot in ((0, h0, ot0), (1, h1, ot1)):
                        lhsT = whaug[jt][:, h, :]
                        for nh in range(2):
                            nc.tensor.matmul(
                                ot[:, nh * 512:(nh + 1) * 512], lhsT,
                                um2[:, ph, nh * 512:(nh + 1) * 512],
                                start=(jt == 0), stop=(jt == NJT - 1))
                        if jt in ACT_JTS:
                            # restore the E2s floor: += [Wh*E2s|E2s]^T @ adj
                            lhsT2 = whe2s[jt][:, h, :]
                            for nh in range(2):
                                nc.tensor.matmul(
                                    ot[:, nh * 512:(nh + 1) * 512], lhsT2,
                                    adj_sb[jt][:, nh * 512:(nh + 1) * 512],
                                    start=False, stop=False)
                last = hp == H // 2 - 1
                finalize(hp, h0, ot0, muls_on_dve=False)
                finalize(hp, h1, ot1, muls_on_dve=last)
                if hp == 1:
                    for it in range(NIT):
                        nc.gpsimd.dma_start(
                            out=out[it * P:(it + 1) * P, 0:256],
                            in_=out_lo[it])
                if hp == 3:
                    for t2 in range(NIT // 2):
                        dst = out[2 * t2 * P:(2 * t2 + 2) * P, 256:512]
                        dst = dst.rearrange("(s p) c -> p s c", p=P)
                        nc.gpsimd.dma_start(out=dst, in_=out_hi[t2])

    nc.compile()
    return nc


def _host_prep(h, adj, W, a):
    """Host-side input prep: transposes / casts / tiny einsums only."""
    a1, a2 = a[:, :D_OUT], a[:, D_OUT:]
    w1 = np.einsum("hdf,hf->hd", W, a1).astype(np.float32)   # [H, D_IN]
    w2 = np.einsum("hdf,hf->hd", W, a2).astype(np.float32)
    w12 = np.zeros((D_IN, FT_W), dtype=np.float32)           # [D_IN, 40]
    w12[:, 0:H] = w1.T
    w12[:, 32:32 + H] = w2.T
    w12 = w12.astype(BF16)
    selm = np.zeros((H, H, P), dtype=np.float32)
    for hh in range(H):
        selm[hh, hh, :] = 1.0
    selm = selm.reshape(H, H * P).astype(BF16)
    wrs = np.ascontiguousarray(
        W.transpose(1, 0, 2).reshape(D_IN, HF)).astype(BF16)
    in_maps = []
    for b in range(B):
        in_maps.append({
            "hT": np.ascontiguousarray(h[b].T).astype(BF16),
            "adjT": np.ascontiguousarray(adj[b].T).astype(BF16),
            "wrs": wrs,
            "w12": w12,
            "sel": selm,
        })
    return in_maps


def kernel(h, adj, W, a):
    from concourse.bass_utils import run_bass_kernel_spmd

    in_maps = _host_prep(np.asarray(h), np.asarray(adj),
                         np.asarray(W), np.asarray(a))
    nc = _build_program()
    res = run_bass_kernel_spmd(nc, in_maps, core_ids=list(range(B)))
    out = np.stack([np.asarray(res.results[b]["out"]) for b in range(B)])
    return out.astype(np.float32)


# revision 40
# speedup vs baseline: 1.1939x; 1.1939x over previous
"""Multi-head graph attention layer (GAT) for Trainium2, 8-core data-parallel.

Problem: B=8, N=1024, D_IN=256, D_OUT=64, H=8, LeakyReLU slope 0.2.
  Wh = einsum('bnd,hdf->bhnf', h, W)
  f1 = Wh @ a1, f2 = Wh @ a2              (per head)
  e  = leaky_relu(f1[:,None] + f2[None,:])
  att = softmax(where(adj==0, -inf, e))
  out = att @ Wh  -> concat heads [B, N, H*F]

Sharding: one batch element per NeuronCore (B=8 across 8 cores).

Algebra: with x = f1_i + f2_j,
  exp(leaky_relu(x)) = max(exp(x), exp(0.2 x))
                     = E1s_i * max(R_i * E2_j, E2s_j)
where R = exp(0.8 f1), E2 = exp(f2), E2s = exp(0.2 f2), E1s = exp(0.2 f1).
The E1s_i factor is constant along j, so it cancels between the softmax
numerator and denominator and is never computed.  Per (head, j-tile) the
[128, N] unnormalized attention U' = max(R_i*E2_j, E2s_j) * adj_ij needs
only TWO elementwise ops:
  1. a dual-op tensor_scalar (mult by per-partition E2_j, max with
     per-partition E2s_j) streaming the broadcast R row — 4x DVE mode;
  2. a tensor_tensor multiply with the adjacency tile, two heads per
     instruction (2x DVE mode).
U' accumulates against [Wh | 1] so column 64 of out^T is the softmax
denominator Z; normalization happens after a PE transpose.

Scheduling notes (from HW traces):
- GPSIMD streaming steals SBUF ports and collapses DVE's 4x
  tensor_scalar mode to 1x (443ns -> 2260ns), so Pool gets no [N,N] work.
- Each dma_start costs ~640ns of serial descriptor-gen on its issuing
  engine and input DMAs compete for HBM bandwidth; input gens are
  serialized on sync in first-use order, output gens on gpsimd.
- The R row broadcast is done by PE outer products against a host-sent
  selector (sel_h @ e12t replicates row h across partitions) + ACT
  evacuation - no DMA round-trip, and it rides the high-priority chain
  that gates the whole DVE stream.
- Tile deps are whole-tile: rall is 8 separate tiles, and the output
  lives in column-half strips so the heads 0-3 half DMAs out mid-run;
  the two drain-head finalizes run concurrently on disjoint engines.
"""

import numpy as np
import ml_dtypes

BF16 = ml_dtypes.bfloat16

B, N, D_IN, D_OUT, H = 8, 1024, 256, 64, 8
NEG_SLOPE = 0.2
P = 128                       # partitions
NJT = N // P                  # 8 j-tiles
NIT = N // P                  # 8 i-tiles
NKT = D_IN // P               # 2 k-tiles
HF = H * D_OUT                # 512
AUG = D_OUT + 1               # 65 (Wh columns + ones column)
FT_W = 40                     # f-score width: f1 cols 0:8, f2 cols 32:40
ACT_JTS = (6,)                 # j-tiles whose max() runs on ACT via the
                              # relu identity (must avoid jt 0 and 7)


def _build_program():
    """Build the single-core SPMD Bass program. Returns nc."""
    import concourse.bass as bass
    import concourse.bacc as bacc
    import concourse.tile as tile
    from concourse import mybir
    from concourse.masks import make_identity

    f32 = mybir.dt.float32
    bf16 = mybir.dt.bfloat16
    AF = mybir.ActivationFunctionType
    OP = mybir.AluOpType

    nc = bacc.Bacc("TRN2", target_bir_lowering=False, debug=False,
                   enable_asserts=False, num_devices=8)

    hT = nc.dram_tensor("hT", [D_IN, N], bf16, kind="ExternalInput").ap()
    adjT = nc.dram_tensor("adjT", [N, N], bf16, kind="ExternalInput").ap()
    wrs = nc.dram_tensor("wrs", [D_IN, HF], bf16, kind="ExternalInput").ap()
    w12 = nc.dram_tensor("w12", [D_IN, FT_W], bf16,
                         kind="ExternalInput").ap()
    sel = nc.dram_tensor("sel", [H, H * P], bf16, kind="ExternalInput").ap()
    out = nc.dram_tensor("out", [N, HF], bf16, kind="ExternalOutput").ap()

    with tile.TileContext(nc) as tc:
        with (
            tc.tile_pool(name="const", bufs=1) as const,
            tc.tile_pool(name="inputs", bufs=1) as inputs,
            tc.tile_pool(name="whp", bufs=1) as whp,
            tc.tile_pool(name="ecol", bufs=1) as ecolp,
            tc.tile_pool(name="ps_s", bufs=2, space="PSUM") as ps_s,
            tc.tile_pool(name="ps_ot", bufs=2, space="PSUM") as ps_ot,
            tc.tile_pool(name="ps_tr", bufs=1, space="PSUM") as ps_tr,
            tc.tile_pool(name="work", bufs=3) as work,
            tc.tile_pool(name="fin", bufs=3) as fin,
        ):
            # ---- Phase 0: input loads -------------------------------------
            # ht/w12 gate the R-broadcast chain: their descriptor-gen goes
            # first on sync.  adj/wrs gen on gpsimd (behind make_identity,
            # which is fine - first use is later).
            # All input descriptor-gen serialized on sync in priority order,
            # so hT's transfers hit clean DMA queues before adj's 2MB.
            ht_sb = []
            for kt in range(NKT):
                t = inputs.tile([P, N], bf16, tag=f"ht{kt}")
                for c in range(2):
                    nc.sync.dma_start(
                        out=t[:, c * 512:(c + 1) * 512],
                        in_=hT[kt * P:(kt + 1) * P, c * 512:(c + 1) * 512])
                ht_sb.append(t)
            w12_sb = []
            for kt in range(NKT):
                t = inputs.tile([P, FT_W], bf16, tag=f"w12{kt}")
                nc.sync.dma_start(out=t, in_=w12[kt * P:(kt + 1) * P, :])
                w12_sb.append(t)
            selsb = const.tile([H, H, P], bf16)
            nc.sync.dma_start(out=selsb, in_=sel)
            adj_sb = []
            for jt in range(NJT):
                t = inputs.tile([P, N], bf16, tag=f"adj{jt}")
                nc.sync.dma_start(out=t, in_=adjT[jt * P:(jt + 1) * P, :])
                adj_sb.append(t)
            wrs_sb = []
            for kt in range(NKT):
                t = inputs.tile([P, HF], bf16, tag=f"wrs{kt}")
                nc.sync.dma_start(out=t, in_=wrs[kt * P:(kt + 1) * P, :])
                wrs_sb.append(t)

            ident = const.tile([P, P], f32)
            make_identity(nc, ident)

            # ---- Phase 1a (critical chain): R broadcast -------------------
            # ft[h, :] = f1_h (transposed);  e12t = exp(0.8 f1)
            with tc.high_priority():
                ft = ps_tr.tile([H, N], f32, tag='big')
                for nh in range(2):
                    for kt in range(NKT):
                        nc.tensor.matmul(ft[:, nh * 512:(nh + 1) * 512],
                                         w12_sb[kt][:, 0:H],
                                         ht_sb[kt][:, nh * 512:(nh + 1) * 512],
                                         start=(kt == 0),
                                         stop=(kt == NKT - 1))
                e12t = const.tile([H, N], bf16)
                nc.scalar.activation(e12t, ft, AF.Exp, scale=1.0 - NEG_SLOPE)
                # broadcast R rows across partitions via PE outer products
                # (ones_row^T @ row) + ACT evacuation - no DMA involved
                rall = []
                for h in range(H):
                    op = ps_tr.tile([P, N], f32, tag='big')
                    for nh in range(2):
                        nc.tensor.matmul(op[:, nh * 512:(nh + 1) * 512],
                                         selsb[:, h, :],
                                         e12t[:, nh * 512:(nh + 1) * 512],
                                         start=True, stop=True)
                    rt = const.tile([P, N], bf16, tag=f"rall{h}")
                    nc.scalar.activation(rt, op, AF.Copy, scale=1.0)
                    rall.append(rt)

            # ---- Phase 1b: per-j-tile scalars E2/E2s ----------------------
            # ecols[jt][:, h] = E2_j = exp(f2_h,j); [:, 32+h] = E2s_j
            ecols = []
            for jt in range(NJT):
                psec = ps_s.tile([P, FT_W], f32, tag='pss')
                for kt in range(NKT):
                    nc.tensor.matmul(psec, ht_sb[kt][:, jt * P:(jt + 1) * P],
                                     w12_sb[kt],
                                     start=(kt == 0), stop=(kt == NKT - 1))
                ec = ecolp.tile([P, FT_W], f32, tag=f"ecols{jt}")
                nc.scalar.activation(ec[:, 0:H], psec[:, 32:32 + H], AF.Exp,
                                     scale=1.0)
                nc.scalar.activation(ec[:, 32:32 + H], psec[:, 32:32 + H],
                                     AF.Exp, scale=NEG_SLOPE)
                if jt in ACT_JTS:   # negated E2s as ACT relu bias
                    nc.vector.tensor_scalar_mul(ec[:, H:2 * H],
                                                ec[:, 32:32 + H], -1.0)
                ecols.append(ec)

            # ---- Phase 2: whaug = [Wh | 1] (no per-head scaling) ----------
            whaug = []
            for jt in range(NJT):
                ps = ps_s.tile([P, HF], f32, tag='pss')
                for kt in range(NKT):
                    lhsT = ht_sb[kt][:, jt * P:(jt + 1) * P]
                    nc.tensor.matmul(ps, lhsT, wrs_sb[kt],
                                     start=(kt == 0), stop=(kt == NKT - 1))
                wa = whp.tile([P, H, AUG], bf16, tag=f"whaug{jt}")
                nc.scalar.activation(wa[:, :, 0:D_OUT], ps, AF.Copy,
                                     scale=1.0)
                nc.vector.memset(wa[:, :, D_OUT], 1.0)
                whaug.append(wa)
            whe2s = {}
            for jt in ACT_JTS:
                we = whp.tile([P, H, AUG], bf16, tag=f"whe{jt}")
                for h in range(H):
                    nc.scalar.activation(we[:, h, :], whaug[jt][:, h, :],
                                         AF.Copy,
                                         scale=ecols[jt][:, 32 + h:33 + h])
                whe2s[jt] = we

            # output strips: column halves per i-tile (512B DMA lines); the
            # heads 0-3 half leaves mid-run, only the hi half is on the tail
            out_lo = []
            for it in range(NIT):
                osl = whp.tile([P, HF // 2], bf16, tag=f"osbl{it}")
                out_lo.append(osl)
            out_hi = []
            for t2 in range(NIT // 2):
                osh = whp.tile([P, 2, HF // 2], bf16, tag=f"osbh{t2}")
                out_hi.append(osh)

            # pre-built ACT-road pt tiles: relu(E2*R - E2s); the E2s*adj
            # remainder is added back by PE matmuls streaming adj directly
            pta = {}
            for hp in range(H // 2):
                for jt in ACT_JTS:
                    t = work.tile([P, 2, N], bf16, tag=f"pta{hp}_{jt}",
                                  bufs=1)
                    for ph in range(2):
                        h = 2 * hp + ph
                        nc.scalar.activation(
                            t[:, ph, :], rall[h], AF.Relu,
                            scale=ecols[jt][:, h:h + 1],
                            bias=ecols[jt][:, H + h:H + h + 1])
                    pta[(hp, jt)] = t

            # ---- Phase 3: per-head-pair attention -------------------------
            def _dst(h, it):
                if h < 4:
                    return out_lo[it][:, h * D_OUT:(h + 1) * D_OUT]
                return out_hi[it // 2][:, it % 2,
                                       (h - 4) * D_OUT:(h - 3) * D_OUT]

            def finalize(hp, h, ot, muls_on_dve):
                ots = fin.tile([AUG, N], f32, tag="ots")
                nc.scalar.copy(ots, ot)
                if muls_on_dve:
                    # drain head: per-it pipeline on pss ring + DVE, fully
                    # concurrent with the sibling head's ACT-road finalize
                    for it in range(NIT):
                        tr2 = ps_s.tile([P, AUG], f32, tag='pss')
                        nc.tensor.transpose(tr2, ots[:, it * P:(it + 1) * P],
                                            ident[0:AUG, 0:AUG])
                        rcol = fin.tile([P, 1], f32, tag="rcol")
                        nc.vector.reciprocal(rcol, tr2[:, D_OUT:AUG])
                        nc.vector.tensor_scalar_mul(_dst(h, it),
                                                    tr2[:, 0:D_OUT], rcol)
                    return
                # stride-128 slots keep each transpose in one PSUM bank
                tr2all = ps_tr.tile([P, NIT, P], f32, tag='big')
                for it in range(NIT):
                    nc.tensor.transpose(tr2all[:, it, 0:AUG],
                                        ots[:, it * P:(it + 1) * P],
                                        ident[0:AUG, 0:AUG])
                rcall = fin.tile([P, NIT], f32, tag="rcall")
                nc.vector.reciprocal(rcall, tr2all[:, :, D_OUT])
                for it in range(NIT):
                    nc.scalar.activation(_dst(h, it), tr2all[:, it, 0:D_OUT],
                                         AF.Copy, scale=rcall[:, it:it + 1])

            for hp in range(H // 2):
                h0, h1 = 2 * hp, 2 * hp + 1
                ot0 = ps_ot.tile([AUG, N], f32, tag="ot")
                ot1 = ps_ot.tile([AUG, N], f32, tag="ot")
                for jt in range(NJT):
                    ec = ecols[jt]
                    if jt in ACT_JTS:
                        pt2 = pta[(hp, jt)]
                    else:
                        # pt = max(R_i*E2_j, E2s_j)  (dual-op ts, 4x DVE)
                        pt2 = work.tile([P, 2, N], bf16, tag="pt", bufs=8)
                        for ph, h in ((0, h0), (1, h1)):
                            nc.vector.tensor_scalar(
                                out=pt2[:, ph, :], in0=rall[h],
                                scalar1=ec[:, h:h + 1],
                                scalar2=ec[:, 32 + h:32 + h + 1],
                                op0=OP.mult, op1=OP.max)
                    # um = pt * adj   (both heads in one 2x DVE pass)
                    um2 = work.tile([P, 2, N], bf16, tag="um", bufs=12)
                    nc.vector.tensor_tensor(
                        out=um2, in0=pt2,
                        in1=adj_sb[jt].unsqueeze(1).broadcast_to([P, 2, N]),
                        op=OP.mult)
                    # out^T[(f|1), i] += [Wh | 1]^T @ um
                    for ph, h, ot in ((0, h0, ot0), (1, h1, ot1)):
                        lhsT = whaug[jt][:, h, :]
                        for nh in range(2):
                            nc.tensor.matmul(
                                ot[:, nh * 512:(nh + 1) * 512], lhsT,
                                um2[:, ph, nh * 512:(nh + 1) * 512],
                                start=(jt == 0), stop=(jt == NJT - 1))
                        if jt in ACT_JTS:
                            # restore the E2s floor: += [Wh*E2s|E2s]^T @ adj
                            lhsT2 = whe2s[jt][:, h, :]
                            for nh in range(2):
                                nc.tensor.matmul(
                                    ot[:, nh * 512:(nh + 1) * 512], lhsT2,
                                    adj_sb[jt][:, nh * 512:(nh + 1) * 512],
                                    start=False, stop=False)
                last = hp == H // 2 - 1
                finalize(hp, h0, ot0, muls_on_dve=False)
                finalize(hp, h1, ot1, muls_on_dve=last)
                if hp == 1:
                    for it in range(NIT):
                        nc.gpsimd.dma_start(
                            out=out[it * P:(it + 1) * P, 0:256],
                            in_=out_lo[it])
                if hp == 3:
                    for t2 in range(NIT // 2):
                        dst = out[2 * t2 * P:(2 * t2 + 2) * P, 256:512]
                        dst = dst.rearrange("(s p) c -> p s c", p=P)
                        nc.gpsimd.dma_start(out=dst, in_=out_hi[t2])

    nc.compile()
    return nc


def _host_prep(h, adj, W, a):
    """Host-side input prep: transposes / casts / tiny einsums only."""
    a1, a2 = a[:, :D_OUT], a[:, D_OUT:]
    w1 = np.einsum("hdf,hf->hd", W, a1).astype(np.float32)   # [H, D_IN]
    w2 = np.einsum("hdf,hf->hd", W, a2).astype(np.float32)
    w12 = np.zeros((D_IN, FT_W), dtype=np.float32)           # [D_IN, 40]
    w12[:, 0:H] = w1.T
    w12[:, 32:32 + H] = w2.T
    w12 = w12.astype(BF16)
    selm = np.zeros((H, H, P), dtype=np.float32)
    for hh in range(H):
        selm[hh, hh, :] = 1.0
    selm = selm.reshape(H, H * P).astype(BF16)
    wrs = np.ascontiguousarray(
        W.transpose(1, 0, 2).reshape(D_IN, HF)).astype(BF16)
    in_maps = []
    for b in range(B):
        in_maps.append({
            "hT": np.ascontiguousarray(h[b].T).astype(BF16),
            "adjT": np.ascontiguousarray(adj[b].T).astype(BF16),
            "wrs": wrs,
            "w12": w12,
            "sel": selm,
        })
    return in_maps


def kernel(h, adj, W, a):
    from concourse.bass_utils import run_bass_kernel_spmd

    in_maps = _host_prep(np.asarray(h), np.asarray(adj),
                         np.asarray(W), np.asarray(a))
    nc = _build_program()
    res = run_bass_kernel_spmd(nc, in_maps, core_ids=list(range(B)))
    out = np.stack([np.asarray(res.results[b]["out"]) for b in range(B)])
    return out.astype(np.float32)
